# revision 19
# baseline (speedup 1.0000x reference)
"""CrissCrossAttention TRN2 kernel.

Sharding: data-parallel over batch B=8 across 8 NeuronCores (one image per
core, no collectives). Per core, with x flattened to [C=512, HW=9216]
(H=W=96):

  q,k = Wq/Wk @ x + b                      (packed -> one [128, HW], fp16)
  E_H[w][j,h] = sum_c k[c,j,w] q[c,h,w]    (per-column, key-major in PSUM)
  E_W[h][j,g] = sum_c k[c,h,j] q[c,h,g]    (per-row, key-major)
  P = exp(E)          (no max-subtraction: |logit| <~ 50 << 88, fp32-exp safe)
  Z[p] = sum_j P_H[j,p] + sum_j P_W[j,p]   (ones-matmuls, PSUM-accumulated)
  y[c,p] = sum_j xT[j,c] (P_H + P_W)[j,p]  (attention applied to x, not v)
  out = Wv @ (y * gamma/Z) + gamma*bv      (Wv folded after attention; softmax
                                            rows sum to 1 so bv passes through)

The wall-clock under the axon client is dominated by host<->device transfer,
so x is shipped ONCE as fp16 [C, HW] (natural layout) and the pixel-major
copy needed as the attention-apply stationary is derived on-device by PE
transposes into an internal DRAM scratch tensor. fp16 matmuls are native on
the PE (exact fp32 accumulation), so the q/k logit path is at least as
accurate as the fp32r emulation it replaces. The output ships int8 with
per-[channel, 512-px window] fp32 scales (dequantized on host), halving
both the D2H bytes and the donated-zero-buffer H2D bytes for ~0.6% added
quantization noise. All PSUM tiles are exactly one 2 KiB bank so no matmul
output ever crosses a bank boundary (HW silently corrupts on crossing --
this was the dominant error source in the previous version).
"""

import numpy as np

B, C, HH, WW = 8, 512, 96, 96
CQ = 64
HW = HH * WW  # 9216
NW = HW // 512  # 18 windows
NCHUNK = C // 128  # 4

_cache = {}


def _build_nc():
    import concourse.mybir as mybir
    import concourse.tile as tile
    from concourse import bacc, masks

    dt = mybir.dt
    AF = mybir.ActivationFunctionType
    ALU = mybir.AluOpType

    nc = bacc.Bacc("TRN2", target_bir_lowering=False, debug=False)

    x_d = nc.dram_tensor("x", [C, HW], dt.float16, kind="ExternalInput")
    wqk_d = nc.dram_tensor("wqkT", [C, 128], dt.float16, kind="ExternalInput")
    wv_d = nc.dram_tensor("wvT", [C, C], dt.float16, kind="ExternalInput")
    bqk_d = nc.dram_tensor("bqk", [128, 1], dt.float32, kind="ExternalInput")
    gbv_d = nc.dram_tensor("gbv", [128, NCHUNK], dt.float32, kind="ExternalInput")
    gam_d = nc.dram_tensor("gam", [1, 1], dt.float32, kind="ExternalInput")
    # int8 block-quantized output (per [channel, 512-px window] fp32 scales):
    # halves the D2H bytes AND the donated-zeros H2C bytes vs fp16 at ~0.5%
    # added quantization noise, well inside the accuracy budget.
    out_d = nc.dram_tensor("out8", [C, HW], dt.int8, kind="ExternalOutput")
    osc_d = nc.dram_tensor("osc", [128, NCHUNK * NW], dt.float32, kind="ExternalOutput")

    with tile.TileContext(nc) as tc:
        with (
            tc.tile_pool(name="dram", bufs=1, space="DRAM") as dram_p,
            tc.tile_pool(name="singles", bufs=1) as singles,
            tc.tile_pool(name="resident", bufs=1) as res,
        ):
            # pixel-major x, derived on-device (phase 0), consumed in phase 4
            xtp = dram_p.tile([HW, C], dt.bfloat16)
            xtp3 = xtp.rearrange("(h w) c -> h w c", w=WW)

            # resident tensors ------------------------------------------------
            pH = res.tile([128, HH, WW], dt.bfloat16)  # exp(E_H): [j, (h,w)]
            pW = res.tile([128, HH, WW], dt.bfloat16)  # exp(E_W): [j=w-idx, (h,w)]
            rbc = res.tile([128, HW], dt.float32)      # gamma/Z on all partitions

            # weights / constants --------------------------------------------
            wqk = singles.tile([128, NCHUNK, 128], dt.float16)
            wv = singles.tile([128, NCHUNK, C], dt.float16)
            for kc in range(NCHUNK):
                nc.sync.dma_start(out=wqk[:, kc, :], in_=wqk_d[128 * kc : 128 * (kc + 1), :])
                nc.sync.dma_start(out=wv[:, kc, :], in_=wv_d[128 * kc : 128 * (kc + 1), :])
            bqk = singles.tile([128, 1], dt.float32)
            nc.sync.dma_start(out=bqk, in_=bqk_d[:, :])
            gbv = singles.tile([128, NCHUNK], dt.float32)
            nc.sync.dma_start(out=gbv, in_=gbv_d[:, :])
            gam = singles.tile([128, 1], dt.float32)
            nc.sync.dma_start(out=gam, in_=gam_d[:, :].to_broadcast([128, 1]))
            ones = singles.tile([128, 128], dt.bfloat16)
            nc.vector.memset(ones, 1.0)
            ident = singles.tile([128, 128], dt.float16)
            masks.make_identity(nc, ident)
            sct = singles.tile([128, NCHUNK, NW], dt.float32)  # quant scales

            pHw = pH.rearrange("p h w -> p w h")  # permuted view (w outer)
            pHf = pH.rearrange("p h w -> p (h w)")
            pWf = pW.rearrange("p h w -> p (h w)")

            # ====== P0+P1: q,k projection + on-device transpose of x ========
            # One pass over x windows feeds both the projection matmuls and
            # the PE transposes that build the pixel-major DRAM copy.
            # PE needs lhsT/rhs at the same partition base, so unpack the
            # [128,512] projection PSUM into separate base-0 q,k tiles via
            # SBUF->SBUF DMA (engines cannot cross partitions; DMA can).
            with tc.tile_pool(name="qkpool", bufs=1) as qk_p:
                qt = qk_p.tile([64, HH, WW], dt.float16)
                kt = qk_p.tile([64, HH, WW], dt.float16)
                qtf = qt.rearrange("p h w -> p (h w)")
                ktf = kt.rearrange("p h w -> p (h w)")
                with (
                    tc.tile_pool(name="xin", bufs=3) as xin_p,
                    tc.tile_pool(name="qksc", bufs=3) as qksc_p,
                    tc.tile_pool(name="qkps", bufs=2, space="PSUM") as qkps_p,
                    tc.tile_pool(name="tpps", bufs=2, space="PSUM") as tpps_p,
                    tc.tile_pool(name="tpsb", bufs=3) as tpsb_p,
                ):
                    for t in range(NW):
                        xt = xin_p.tile([128, NCHUNK, 512], dt.float16, tag="xin")
                        for kc in range(NCHUNK):
                            nc.sync.dma_start(
                                out=xt[:, kc, :],
                                in_=x_d[128 * kc : 128 * (kc + 1), 512 * t : 512 * (t + 1)],
                            )
                        ps = qkps_p.tile([128, 512], dt.float32, tag="qkps")
                        for kc in range(NCHUNK):
                            nc.tensor.matmul(
                                ps, wqk[:, kc, :], xt[:, kc, :],
                                start=(kc == 0), stop=(kc == NCHUNK - 1),
                            )
                        sc = qksc_p.tile([128, 512], dt.float16, tag="qksc")
                        nc.scalar.activation(
                            out=sc, in_=ps, func=AF.Identity, bias=bqk, scale=1.0,
                        )
                        nc.sync.dma_start(
                            out=qtf[:, 512 * t : 512 * (t + 1)], in_=sc[0:64, :]
                        )
                        nc.sync.dma_start(
                            out=ktf[:, 512 * t : 512 * (t + 1)], in_=sc[64:128, :]
                        )
                        # transpose this window's 4x 128-pixel groups
                        for sub in range(4):
                            pt = tpps_p.tile([128, 512], dt.float16, tag="tpps")
                            for kc in range(NCHUNK):
                                nc.tensor.transpose(
                                    pt[:, 128 * kc : 128 * (kc + 1)],
                                    xt[:, kc, 128 * sub : 128 * (sub + 1)],
                                    ident,
                                )
                            ot = tpsb_p.tile([128, 512], dt.bfloat16, tag="tpsb")
                            nc.scalar.activation(out=ot, in_=pt, func=AF.Copy)
                            nc.sync.dma_start(
                                out=xtp[512 * t + 128 * sub : 512 * t + 128 * (sub + 1), :],
                                in_=ot,
                            )

                # ================= P2: energies + exp =======================
                # 4 cols per PSUM tile: [128, 4, 128] fp32 is exactly one
                # 2 KiB bank, so the [.,i,0:96] matmul outputs never cross a
                # bank boundary (HW silently corrupts on crossing).
                with tc.tile_pool(name="eps", bufs=3, space="PSUM") as eps_p:
                    # column attention
                    for wg in range(WW // 4):
                        ep = eps_p.tile([128, 4, 128], dt.float32, tag="eps")
                        for i in range(4):
                            w = 4 * wg + i
                            nc.tensor.matmul(
                                ep[0:96, i, 0:96],
                                kt[:, :, w], qt[:, :, w],
                                start=True, stop=True, tile_position=(0, 0),
                            )
                        nc.scalar.activation(
                            out=pHw[0:96, 4 * wg : 4 * wg + 4, :],
                            in_=ep[0:96, :, 0:96],
                            func=AF.Exp,
                        )
                    # row attention
                    for hg in range(HH // 4):
                        ep = eps_p.tile([128, 4, 128], dt.float32, tag="eps")
                        for i in range(4):
                            h = 4 * hg + i
                            nc.tensor.matmul(
                                ep[0:96, i, 0:96],
                                kt[:, h, :], qt[:, h, :],
                                start=True, stop=True, tile_position=(0, 0),
                            )
                        nc.scalar.activation(
                            out=pW[0:96, 4 * hg : 4 * hg + 4, :],
                            in_=ep[0:96, :, 0:96],
                            func=AF.Exp,
                        )

            # ================= P3: softmax denominators =====================
            # Z broadcast to all 128 partitions via M=128 ones-stationary.
            with (
                tc.tile_pool(name="dps", bufs=2, space="PSUM") as dps_p,
                tc.tile_pool(name="dsc", bufs=3) as dsc_p,
            ):
                for t in range(NW):
                    sl = slice(512 * t, 512 * (t + 1))
                    dpH = dps_p.tile([128, 512], dt.float32, tag="dpH")
                    dpW = dps_p.tile([128, 512], dt.float32, tag="dpW")
                    nc.tensor.matmul(
                        dpH, ones[0:96, :], pHf[0:96, sl],
                        start=True, stop=True, tile_position=(0, 0),
                    )
                    nc.tensor.matmul(
                        dpW, ones[0:96, :], pWf[0:96, sl],
                        start=True, stop=True, tile_position=(0, 0),
                    )
                    sc = dsc_p.tile([128, 512], dt.float32, tag="dsc")
                    nc.scalar.activation(out=sc, in_=dpH, func=AF.Copy)
                    nc.vector.tensor_tensor(out=sc, in0=dpW, in1=sc, op=ALU.add)
                    nc.vector.reciprocal(out=rbc[:, sl], in_=sc)
                    nc.vector.tensor_scalar_mul(
                        out=rbc[:, sl], in0=rbc[:, sl], scalar1=gam
                    )

            # ================= P4: attention apply ==========================
            with (
                tc.tile_pool(name="ypool", bufs=1) as y_p,
                tc.tile_pool(name="xc", bufs=4) as xc_p,
                tc.tile_pool(name="yps", bufs=3, space="PSUM") as yps_p,
            ):
                y = y_p.tile([128, NCHUNK, HW], dt.bfloat16)
                y4 = y.rearrange("p c (h w) -> p c h w", w=WW)
                # columns: y[c, (h,w)] for fixed w
                for w in range(WW):
                    xc = xc_p.tile([96, C], dt.bfloat16, tag="xc")
                    nc.sync.dma_start(out=xc, in_=xtp3[:, w, :])
                    yp = yps_p.tile([128, NCHUNK, 128], dt.float32, tag="yps")
                    for kc in range(NCHUNK):
                        nc.tensor.matmul(
                            yp[:, kc, 0:96],
                            xc[:, 128 * kc : 128 * (kc + 1)],
                            pH[0:96, :, w],
                            start=True, stop=True,
                        )
                    nc.scalar.activation(
                        out=y4[:, :, :, w], in_=yp[:, :, 0:96], func=AF.Copy,
                    )
                # rows: accumulate into y
                for h in range(HH):
                    xc = xc_p.tile([96, C], dt.bfloat16, tag="xc")
                    nc.sync.dma_start(out=xc, in_=xtp3[h, :, :])
                    yp = yps_p.tile([128, NCHUNK, 128], dt.float32, tag="yps")
                    for kc in range(NCHUNK):
                        nc.tensor.matmul(
                            yp[:, kc, 0:96],
                            xc[:, 128 * kc : 128 * (kc + 1)],
                            pW[0:96, h, :],
                            start=True, stop=True,
                        )
                    ysl = y[:, :, 96 * h : 96 * (h + 1)]
                    nc.vector.tensor_tensor(
                        out=ysl, in0=yp[:, :, 0:96], in1=ysl, op=ALU.add
                    )

                # ============ P5+P6: normalize + Wv apply + bias ============
                with (
                    tc.tile_pool(name="ynst", bufs=2) as yn_p,
                    tc.tile_pool(name="ops", bufs=2, space="PSUM") as ops_p,
                    tc.tile_pool(name="osb", bufs=3) as osb_p,
                    tc.tile_pool(name="mxp", bufs=4) as mx_p,
                    tc.tile_pool(name="q8p", bufs=3) as q8_p,
                ):
                    for t in range(NW):
                        sl = slice(512 * t, 512 * (t + 1))
                        yn = yn_p.tile([128, NCHUNK, 512], dt.float16, tag="yn")
                        with nc.allow_low_precision(
                            reason="normalized values are O(1); fp16 is ample"
                        ):
                            for kc in range(NCHUNK):
                                nc.vector.tensor_tensor(
                                    out=yn[:, kc, :], in0=y[:, kc, sl],
                                    in1=rbc[:, sl], op=ALU.mult,
                                )
                        for oc in range(NCHUNK):
                            op = ops_p.tile([128, 512], dt.float32, tag="ops")
                            for kc in range(NCHUNK):
                                nc.tensor.matmul(
                                    op,
                                    wv[:, kc, 128 * oc : 128 * (oc + 1)],
                                    yn[:, kc, :],
                                    start=(kc == 0), stop=(kc == NCHUNK - 1),
                                )
                            ot = osb_p.tile([128, 512], dt.float32, tag="osb")
                            nc.scalar.activation(
                                out=ot, in_=op, func=AF.Identity,
                                bias=gbv[:, oc : oc + 1], scale=1.0,
                            )
                            mx = mx_p.tile([128, 1], dt.float32, tag="mx")
                            nc.vector.tensor_reduce(
                                out=mx, in_=ot, axis=mybir.AxisListType.X,
                                op=ALU.max, apply_absolute_value=True,
                            )
                            nc.vector.tensor_scalar_max(
                                out=mx, in0=mx, scalar1=1e-20
                            )
                            nc.vector.tensor_scalar_mul(
                                out=sct[:, oc, t : t + 1], in0=mx,
                                scalar1=1.0 / 127.0,
                            )
                            inv = mx_p.tile([128, 1], dt.float32, tag="inv")
                            nc.vector.reciprocal(out=inv, in_=mx)
                            nc.vector.tensor_scalar_mul(
                                out=inv, in0=inv, scalar1=127.0
                            )
                            # HW fp32->int8 conversion rounds to nearest
                            # (CoreSim truncates -- sim/HW divergence; the HW
                            # result is truth: measured 8.5e-3 vs the 1.66e-2
                            # a truncating convert would give).
                            q8 = q8_p.tile([128, 512], dt.int8, tag="q8")
                            with nc.allow_low_precision(
                                reason="int8 block quantization of the output"
                            ):
                                nc.vector.tensor_scalar_mul(
                                    out=q8, in0=ot, scalar1=inv
                                )
                            nc.sync.dma_start(
                                out=out_d[128 * oc : 128 * (oc + 1), sl], in_=q8
                            )
                    nc.sync.dma_start(
                        out=osc_d[:, :], in_=sct.rearrange("p a b -> p (a b)")
                    )

    nc.finalize()
    return nc


def _get_nc():
    if "nc" not in _cache:
        _cache["nc"] = _build_nc()
    return _cache["nc"]


def _prep_maps(x, Wq, bq, Wk, bk, Wv, bv, gamma):
    x = np.asarray(x)
    wqkT = np.concatenate(
        [np.asarray(Wq).T, np.asarray(Wk).T], axis=1
    ).astype(np.float16)  # [512, 128]
    wvT = np.ascontiguousarray(np.asarray(Wv).T).astype(np.float16)
    bqk = (
        np.concatenate([np.asarray(bq), np.asarray(bk)])
        .reshape(128, 1)
        .astype(np.float32)
    )
    g = float(np.asarray(gamma).reshape(-1)[0])
    gbv = (g * np.asarray(bv)).reshape(NCHUNK, 128).T.copy().astype(np.float32)
    gam = np.full((1, 1), g, dtype=np.float32)

    in_maps = []
    for b in range(B):
        xb = x[b].reshape(C, HW).astype(np.float16)
        in_maps.append(
            {
                "x": xb,
                "wqkT": wqkT,
                "wvT": wvT,
                "bqk": bqk,
                "gbv": gbv,
                "gam": gam,
            }
        )
    return in_maps


def _dequant(o8, osc):
    # o8: [C, HW] int8, row c = 128*oc + p; osc: [128, NCHUNK*NW] fp32 with
    # scale for (channel p of chunk oc, 512-px window t) at [p, oc*NW + t].
    q = o8.astype(np.float32).reshape(NCHUNK, 128, NW, 512)
    s = osc.reshape(128, NCHUNK, NW).transpose(1, 0, 2)[:, :, :, None]
    return (q * s).reshape(C, HH, WW)


def kernel(x, Wq, bq, Wk, bk, Wv, bv, gamma):
    from concourse.bass_utils import run_bass_kernel_spmd

    in_maps = _prep_maps(x, Wq, bq, Wk, bk, Wv, bv, gamma)
    res = run_bass_kernel_spmd(_get_nc(), in_maps, core_ids=list(range(B)))
    return np.stack(
        [_dequant(res.results[b]["out8"], res.results[b]["osc"]) for b in range(B)]
    )


# revision 20
# speedup vs baseline: 1.1236x; 1.1236x over previous
"""CrissCrossAttention TRN2 kernel.

Sharding: data-parallel over batch B=8 across 8 NeuronCores (one image per
core, no collectives). Per core, with x flattened to [C=512, HW=9216]
(H=W=96):

  q,k = Wq/Wk @ x + b                      (packed -> one [128, HW], fp16)
  E_H[w][j,h] = sum_c k[c,j,w] q[c,h,w]    (per-column, key-major in PSUM)
  E_W[h][j,g] = sum_c k[c,h,j] q[c,h,g]    (per-row, key-major)
  P = exp(E)          (no max-subtraction: |logit| <~ 50 << 88, fp32-exp safe)
  Z[p] = sum_j P_H[j,p] + sum_j P_W[j,p]   (ones-matmuls, PSUM-accumulated)
  y[c,p] = sum_j xT[j,c] (P_H + P_W)[j,p]  (attention applied to x, not v)
  out = Wv @ (y * gamma/Z) + gamma*bv      (Wv folded after attention; softmax
                                            rows sum to 1 so bv passes through)

The wall-clock under the axon client is dominated by host<->device transfer,
so x is shipped ONCE as fp16 [C, HW] (natural layout) and the pixel-major
copy needed as the attention-apply stationary is derived on-device by PE
transposes into an internal DRAM scratch tensor. fp16 matmuls are native on
the PE (exact fp32 accumulation), so the q/k logit path is at least as
accurate as the fp32r emulation it replaces. The output ships int8 with
per-[channel, 512-px window] fp32 scales (dequantized on host), halving
both the D2H bytes and the donated-zero-buffer H2D bytes for ~0.6% added
quantization noise. All PSUM tiles are exactly one 2 KiB bank so no matmul
output ever crosses a bank boundary (HW silently corrupts on crossing --
this was the dominant error source in the previous version).
"""

import numpy as np

B, C, HH, WW = 8, 512, 96, 96
CQ = 64
HW = HH * WW  # 9216
NW = HW // 512  # 18 windows
NCHUNK = C // 128  # 4

_cache = {}


def _build_nc():
    import concourse.mybir as mybir
    import concourse.tile as tile
    from concourse import bacc, masks

    dt = mybir.dt
    AF = mybir.ActivationFunctionType
    ALU = mybir.AluOpType

    nc = bacc.Bacc("TRN2", target_bir_lowering=False, debug=False)

    x_d = nc.dram_tensor("x", [C, HW], dt.float16, kind="ExternalInput")
    wqk_d = nc.dram_tensor("wqkT", [C, 128], dt.float16, kind="ExternalInput")
    wv_d = nc.dram_tensor("wvT", [C, C], dt.float16, kind="ExternalInput")
    bqk_d = nc.dram_tensor("bqk", [128, 1], dt.float32, kind="ExternalInput")
    gbv_d = nc.dram_tensor("gbv", [128, NCHUNK], dt.float32, kind="ExternalInput")
    gam_d = nc.dram_tensor("gam", [1, 1], dt.float32, kind="ExternalInput")
    # int8 block-quantized output (per [channel, 512-px window] fp32 scales):
    # halves the D2H bytes AND the donated-zeros H2C bytes vs fp16 at ~0.5%
    # added quantization noise, well inside the accuracy budget.
    out_d = nc.dram_tensor("out8", [C, HW], dt.int8, kind="ExternalOutput")
    osc_d = nc.dram_tensor("osc", [128, NCHUNK * NW], dt.float32, kind="ExternalOutput")

    with tile.TileContext(nc) as tc:
        with (
            tc.tile_pool(name="dram", bufs=1, space="DRAM") as dram_p,
            tc.tile_pool(name="singles", bufs=1) as singles,
            tc.tile_pool(name="resident", bufs=1) as res,
        ):
            # pixel-major x, derived on-device (phase 0), consumed in phase 4
            xtp = dram_p.tile([HW, C], dt.bfloat16)
            xtp3 = xtp.rearrange("(h w) c -> h w c", w=WW)

            # resident tensors ------------------------------------------------
            pH = res.tile([128, HH, WW], dt.bfloat16)  # exp(E_H): [j, (h,w)]
            pW = res.tile([128, HH, WW], dt.bfloat16)  # exp(E_W): [j=w-idx, (h,w)]
            rbc = res.tile([128, HW], dt.float32)      # gamma/Z on all partitions

            # weights / constants --------------------------------------------
            wqk = singles.tile([128, NCHUNK, 128], dt.float16)
            wv = singles.tile([128, NCHUNK, C], dt.float16)
            for kc in range(NCHUNK):
                nc.sync.dma_start(out=wqk[:, kc, :], in_=wqk_d[128 * kc : 128 * (kc + 1), :])
                nc.sync.dma_start(out=wv[:, kc, :], in_=wv_d[128 * kc : 128 * (kc + 1), :])
            bqk = singles.tile([128, 1], dt.float32)
            nc.sync.dma_start(out=bqk, in_=bqk_d[:, :])
            gbv = singles.tile([128, NCHUNK], dt.float32)
            nc.sync.dma_start(out=gbv, in_=gbv_d[:, :])
            gam = singles.tile([128, 1], dt.float32)
            nc.sync.dma_start(out=gam, in_=gam_d[:, :].to_broadcast([128, 1]))
            ones = singles.tile([128, 128], dt.bfloat16)
            nc.vector.memset(ones, 1.0)
            ident = singles.tile([128, 128], dt.float16)
            masks.make_identity(nc, ident)
            sct = singles.tile([128, NCHUNK, NW], dt.float32)  # quant scales

            pHw = pH.rearrange("p h w -> p w h")  # permuted view (w outer)
            pHf = pH.rearrange("p h w -> p (h w)")
            pWf = pW.rearrange("p h w -> p (h w)")

            # ====== P0+P1: q,k projection + on-device transpose of x ========
            # One pass over x windows feeds both the projection matmuls and
            # the PE transposes that build the pixel-major DRAM copy.
            # PE needs lhsT/rhs at the same partition base, so unpack the
            # [128,512] projection PSUM into separate base-0 q,k tiles via
            # SBUF->SBUF DMA (engines cannot cross partitions; DMA can).
            with tc.tile_pool(name="qkpool", bufs=1) as qk_p:
                qt = qk_p.tile([64, HH, WW], dt.float16)
                kt = qk_p.tile([64, HH, WW], dt.float16)
                qtf = qt.rearrange("p h w -> p (h w)")
                ktf = kt.rearrange("p h w -> p (h w)")
                with (
                    tc.tile_pool(name="xin", bufs=3) as xin_p,
                    tc.tile_pool(name="qksc", bufs=3) as qksc_p,
                    tc.tile_pool(name="qkps", bufs=2, space="PSUM") as qkps_p,
                    tc.tile_pool(name="tpps", bufs=2, space="PSUM") as tpps_p,
                    tc.tile_pool(name="tpsb", bufs=3) as tpsb_p,
                ):
                    for t in range(NW):
                        xt = xin_p.tile([128, NCHUNK, 512], dt.float16, tag="xin")
                        for kc in range(NCHUNK):
                            nc.sync.dma_start(
                                out=xt[:, kc, :],
                                in_=x_d[128 * kc : 128 * (kc + 1), 512 * t : 512 * (t + 1)],
                            )
                        ps = qkps_p.tile([128, 512], dt.float32, tag="qkps")
                        for kc in range(NCHUNK):
                            nc.tensor.matmul(
                                ps, wqk[:, kc, :], xt[:, kc, :],
                                start=(kc == 0), stop=(kc == NCHUNK - 1),
                            )
                        sc = qksc_p.tile([128, 512], dt.float16, tag="qksc")
                        nc.scalar.activation(
                            out=sc, in_=ps, func=AF.Identity, bias=bqk, scale=1.0,
                        )
                        nc.sync.dma_start(
                            out=qtf[:, 512 * t : 512 * (t + 1)], in_=sc[0:64, :]
                        )
                        nc.sync.dma_start(
                            out=ktf[:, 512 * t : 512 * (t + 1)], in_=sc[64:128, :]
                        )
                        # transpose this window's 4x 128-pixel groups
                        for sub in range(4):
                            pt = tpps_p.tile([128, 512], dt.float16, tag="tpps")
                            for kc in range(NCHUNK):
                                nc.tensor.transpose(
                                    pt[:, 128 * kc : 128 * (kc + 1)],
                                    xt[:, kc, 128 * sub : 128 * (sub + 1)],
                                    ident,
                                )
                            ot = tpsb_p.tile([128, 512], dt.bfloat16, tag="tpsb")
                            nc.scalar.activation(out=ot, in_=pt, func=AF.Copy)
                            nc.sync.dma_start(
                                out=xtp[512 * t + 128 * sub : 512 * t + 128 * (sub + 1), :],
                                in_=ot,
                            )

                # ================= P2: energies + exp =======================
                # 4 cols per PSUM tile: [128, 4, 128] fp32 is exactly one
                # 2 KiB bank, so the [.,i,0:96] matmul outputs never cross a
                # bank boundary (HW silently corrupts on crossing).
                with tc.tile_pool(name="eps", bufs=3, space="PSUM") as eps_p:
                    # column attention
                    for wg in range(WW // 4):
                        ep = eps_p.tile([128, 4, 128], dt.float32, tag="eps")
                        for i in range(4):
                            w = 4 * wg + i
                            nc.tensor.matmul(
                                ep[0:96, i, 0:96],
                                kt[:, :, w], qt[:, :, w],
                                start=True, stop=True, tile_position=(0, 0),
                            )
                        nc.scalar.activation(
                            out=pHw[0:96, 4 * wg : 4 * wg + 4, :],
                            in_=ep[0:96, :, 0:96],
                            func=AF.Exp,
                        )
                    # row attention
                    for hg in range(HH // 4):
                        ep = eps_p.tile([128, 4, 128], dt.float32, tag="eps")
                        for i in range(4):
                            h = 4 * hg + i
                            nc.tensor.matmul(
                                ep[0:96, i, 0:96],
                                kt[:, h, :], qt[:, h, :],
                                start=True, stop=True, tile_position=(0, 0),
                            )
                        nc.scalar.activation(
                            out=pW[0:96, 4 * hg : 4 * hg + 4, :],
                            in_=ep[0:96, :, 0:96],
                            func=AF.Exp,
                        )

            # ================= P3: softmax denominators =====================
            # Z broadcast to all 128 partitions via M=128 ones-stationary.
            with (
                tc.tile_pool(name="dps", bufs=2, space="PSUM") as dps_p,
                tc.tile_pool(name="dsc", bufs=3) as dsc_p,
            ):
                for t in range(NW):
                    sl = slice(512 * t, 512 * (t + 1))
                    dpH = dps_p.tile([128, 512], dt.float32, tag="dpH")
                    dpW = dps_p.tile([128, 512], dt.float32, tag="dpW")
                    nc.tensor.matmul(
                        dpH, ones[0:96, :], pHf[0:96, sl],
                        start=True, stop=True, tile_position=(0, 0),
                    )
                    nc.tensor.matmul(
                        dpW, ones[0:96, :], pWf[0:96, sl],
                        start=True, stop=True, tile_position=(0, 0),
                    )
                    sc = dsc_p.tile([128, 512], dt.float32, tag="dsc")
                    nc.scalar.activation(out=sc, in_=dpH, func=AF.Copy)
                    nc.vector.tensor_tensor(out=sc, in0=dpW, in1=sc, op=ALU.add)
                    nc.vector.reciprocal(out=rbc[:, sl], in_=sc)
                    nc.vector.tensor_scalar_mul(
                        out=rbc[:, sl], in0=rbc[:, sl], scalar1=gam
                    )

            # ================= P4: attention apply ==========================
            with (
                tc.tile_pool(name="ypool", bufs=1) as y_p,
                tc.tile_pool(name="xc", bufs=4) as xc_p,
                tc.tile_pool(name="yps", bufs=3, space="PSUM") as yps_p,
            ):
                y = y_p.tile([128, NCHUNK, HW], dt.bfloat16)
                y4 = y.rearrange("p c (h w) -> p c h w", w=WW)
                # columns: y[c, (h,w)] for fixed w
                for w in range(WW):
                    xc = xc_p.tile([96, C], dt.bfloat16, tag="xc")
                    nc.sync.dma_start(out=xc, in_=xtp3[:, w, :])
                    yp = yps_p.tile([128, NCHUNK, 128], dt.float32, tag="yps")
                    for kc in range(NCHUNK):
                        nc.tensor.matmul(
                            yp[:, kc, 0:96],
                            xc[:, 128 * kc : 128 * (kc + 1)],
                            pH[0:96, :, w],
                            start=True, stop=True,
                        )
                    nc.scalar.activation(
                        out=y4[:, :, :, w], in_=yp[:, :, 0:96], func=AF.Copy,
                    )
                # rows: accumulate into y
                for h in range(HH):
                    xc = xc_p.tile([96, C], dt.bfloat16, tag="xc")
                    nc.sync.dma_start(out=xc, in_=xtp3[h, :, :])
                    yp = yps_p.tile([128, NCHUNK, 128], dt.float32, tag="yps")
                    for kc in range(NCHUNK):
                        nc.tensor.matmul(
                            yp[:, kc, 0:96],
                            xc[:, 128 * kc : 128 * (kc + 1)],
                            pW[0:96, h, :],
                            start=True, stop=True,
                        )
                    ysl = y[:, :, 96 * h : 96 * (h + 1)]
                    nc.vector.tensor_tensor(
                        out=ysl, in0=yp[:, :, 0:96], in1=ysl, op=ALU.add
                    )

                # ============ P5+P6: normalize + Wv apply + bias ============
                with (
                    tc.tile_pool(name="ynst", bufs=2) as yn_p,
                    tc.tile_pool(name="ops", bufs=2, space="PSUM") as ops_p,
                    tc.tile_pool(name="osb", bufs=3) as osb_p,
                    tc.tile_pool(name="mxp", bufs=4) as mx_p,
                    tc.tile_pool(name="q8p", bufs=3) as q8_p,
                ):
                    for t in range(NW):
                        sl = slice(512 * t, 512 * (t + 1))
                        yn = yn_p.tile([128, NCHUNK, 512], dt.float16, tag="yn")
                        with nc.allow_low_precision(
                            reason="normalized values are O(1); fp16 is ample"
                        ):
                            for kc in range(NCHUNK):
                                nc.vector.tensor_tensor(
                                    out=yn[:, kc, :], in0=y[:, kc, sl],
                                    in1=rbc[:, sl], op=ALU.mult,
                                )
                        for oc in range(NCHUNK):
                            op = ops_p.tile([128, 512], dt.float32, tag="ops")
                            for kc in range(NCHUNK):
                                nc.tensor.matmul(
                                    op,
                                    wv[:, kc, 128 * oc : 128 * (oc + 1)],
                                    yn[:, kc, :],
                                    start=(kc == 0), stop=(kc == NCHUNK - 1),
                                )
                            ot = osb_p.tile([128, 512], dt.float32, tag="osb")
                            nc.scalar.activation(
                                out=ot, in_=op, func=AF.Identity,
                                bias=gbv[:, oc : oc + 1], scale=1.0,
                            )
                            mx = mx_p.tile([128, 1], dt.float32, tag="mx")
                            nc.vector.tensor_reduce(
                                out=mx, in_=ot, axis=mybir.AxisListType.X,
                                op=ALU.max, apply_absolute_value=True,
                            )
                            nc.vector.tensor_scalar_max(
                                out=mx, in0=mx, scalar1=1e-20
                            )
                            nc.vector.tensor_scalar_mul(
                                out=sct[:, oc, t : t + 1], in0=mx,
                                scalar1=1.0 / 127.0,
                            )
                            inv = mx_p.tile([128, 1], dt.float32, tag="inv")
                            nc.vector.reciprocal(out=inv, in_=mx)
                            nc.vector.tensor_scalar_mul(
                                out=inv, in0=inv, scalar1=127.0
                            )
                            # HW fp32->int8 conversion rounds to nearest
                            # (CoreSim truncates -- sim/HW divergence; the HW
                            # result is truth: measured 8.5e-3 vs the 1.66e-2
                            # a truncating convert would give).
                            q8 = q8_p.tile([128, 512], dt.int8, tag="q8")
                            with nc.allow_low_precision(
                                reason="int8 block quantization of the output"
                            ):
                                nc.vector.tensor_scalar_mul(
                                    out=q8, in0=ot, scalar1=inv
                                )
                            nc.sync.dma_start(
                                out=out_d[128 * oc : 128 * (oc + 1), sl], in_=q8
                            )
                    nc.sync.dma_start(
                        out=osc_d[:, :], in_=sct.rearrange("p a b -> p (a b)")
                    )

    nc.finalize()
    return nc


def _enable_jax_compilation_cache():
    # The axon redirect inside run_bass_kernel_spmd re-jits a fresh closure
    # per call, so the in-memory jit cache always misses and every dispatch
    # pays XLA compile + the neuronx custom-call hook (~0.6-0.9 s). The
    # persistent cache is keyed on HLO, which IS identical across calls.
    if "jaxcache" in _cache:
        return
    _cache["jaxcache"] = True
    try:
        import jax

        jax.config.update("jax_compilation_cache_dir", "/tmp/jax_comp_cache")
        jax.config.update("jax_persistent_cache_min_compile_time_secs", 0)
        jax.config.update("jax_persistent_cache_min_entry_size_bytes", 0)
    except Exception:
        pass


def _get_nc():
    if "nc" not in _cache:
        _cache["nc"] = _build_nc()
    _enable_jax_compilation_cache()
    return _cache["nc"]


def _prep_maps(x, Wq, bq, Wk, bk, Wv, bv, gamma):
    x = np.asarray(x)
    wqkT = np.concatenate(
        [np.asarray(Wq).T, np.asarray(Wk).T], axis=1
    ).astype(np.float16)  # [512, 128]
    wvT = np.ascontiguousarray(np.asarray(Wv).T).astype(np.float16)
    bqk = (
        np.concatenate([np.asarray(bq), np.asarray(bk)])
        .reshape(128, 1)
        .astype(np.float32)
    )
    g = float(np.asarray(gamma).reshape(-1)[0])
    gbv = (g * np.asarray(bv)).reshape(NCHUNK, 128).T.copy().astype(np.float32)
    gam = np.full((1, 1), g, dtype=np.float32)

    in_maps = []
    for b in range(B):
        xb = x[b].reshape(C, HW).astype(np.float16)
        in_maps.append(
            {
                "x": xb,
                "wqkT": wqkT,
                "wvT": wvT,
                "bqk": bqk,
                "gbv": gbv,
                "gam": gam,
            }
        )
    return in_maps


def _dequant(o8, osc):
    # o8: [C, HW] int8, row c = 128*oc + p; osc: [128, NCHUNK*NW] fp32 with
    # scale for (channel p of chunk oc, 512-px window t) at [p, oc*NW + t].
    q = o8.astype(np.float32).reshape(NCHUNK, 128, NW, 512)
    s = osc.reshape(128, NCHUNK, NW).transpose(1, 0, 2)[:, :, :, None]
    return (q * s).reshape(C, HH, WW)


def kernel(x, Wq, bq, Wk, bk, Wv, bv, gamma):
    from concourse.bass_utils import run_bass_kernel_spmd

    in_maps = _prep_maps(x, Wq, bq, Wk, bk, Wv, bv, gamma)
    res = run_bass_kernel_spmd(_get_nc(), in_maps, core_ids=list(range(B)))
    return np.stack(
        [_dequant(res.results[b]["out8"], res.results[b]["osc"]) for b in range(B)]
    )
